# revision 1
# baseline (speedup 1.0000x reference)
"""Trainium2 Bass kernel for ConvMosaic: 3x3 conv (pad 1) where the weight set
depends on output position p%16 == w%16 (column phase).

Strategy (8 NeuronCores, SPMD):
  - Shard over (N, H): core k handles image k//2, row-half k%2 (128 rows).
  - Host pre-pads each core chunk to [C=32, 130, 258] (halo rows + zero cols).
  - On-chip: x staged as 3 row-shifted partition blocks [(di,c)=96, 32h, 258w]
    so one matmul contracts K=96=(di,c); the dj taps are free-dim offsets.
  - Per phase s: PSUM tile [64oc, 32h, 16w16]; 3 accumulating fp32r matmuls
    (F=512 -> full-rate fp32r); evacuate with alternating DVE/ACT strided
    copies into a contiguous [64, 32, 256] output staging tile; big DMAs out.
"""

import sys

import numpy as np

for _p in ("/opt/trn_rl_repo",):
    if _p not in sys.path:
        sys.path.insert(0, _p)

N, C, H, W = 4, 32, 256, 256
OC = 64
SPE = 16
KDIM = 96  # (di, c) contraction partitions
NCORES = 8
ROWS = H * N // NCORES  # 128 rows per core
GR = 32  # rows per group
GROUPS = ROWS // GR

_cache = {}


def build_nc():
    import bass_rust
    from concourse import bacc, bass, mybir, tile

    f32 = mybir.dt.float32
    f32r = mybir.dt.float32r

    nc = bacc.Bacc()
    xin = nc.dram_tensor("xin", [C, ROWS + 2, W + 2], f32r, kind="ExternalInput")
    wdr = nc.dram_tensor("w", [KDIM, SPE, 3, OC], f32r, kind="ExternalInput")
    out = nc.dram_tensor("out", [OC, ROWS, W], f32, kind="ExternalOutput")

    with tile.TileContext(nc) as tc:
        with (
            tc.tile_pool(name="wp", bufs=1) as wp,
            tc.tile_pool(name="xp", bufs=3) as xp,
            tc.tile_pool(name="op", bufs=2) as op,
            tc.tile_pool(name="ppA", bufs=2, space=bass.MemorySpace.PSUM) as ppA,
            tc.tile_pool(name="ppB", bufs=2, space=bass.MemorySpace.PSUM) as ppB,
        ):
            w_sb = wp.tile([KDIM, SPE, 3, OC], f32r)

            x3s = []
            for g in range(GROUPS):
                x3 = xp.tile([KDIM, GR, W + 2], f32r)
                # single DMA: src AP iterates (di, c, h, ww) with overlapping
                # row windows (di stride = 1 row) -> fills all 96 partitions
                src = xin[:, g * GR : g * GR + GR + 2, :]
                src = src.unsqueeze(0).broadcast_to([3, C, GR + 2, W + 2])
                src.ap = bass_rust.VecI64Pair(
                    [
                        [W + 2, 3],
                        [(ROWS + 2) * (W + 2), C],
                        [W + 2, GR],
                        [1, W + 2],
                    ]
                )
                if g == 0:
                    nc.sync.dma_start(x3[:], src)
                    nc.sync.dma_start(w_sb[:], wdr[:])
                    # dummy matmul: absorbs the weight-DMA wait so the first
                    # real matmul carries only one wait (self-loading fp32r
                    # matmuls support a single sync wait).
                    scratch = ppA.tile([OC, 16], f32, tag="ps")
                    nc.tensor.matmul(
                        scratch[:],
                        w_sb[:, 0, 0, :],
                        w_sb[:, 0, 0, 0:16],
                        start=True,
                        stop=True,
                    )
                else:
                    nc.sync.dma_start(x3[:], src)
                x3s.append(x3)

            for g in range(GROUPS):
                x3 = x3s[g]
                o_sb = op.tile([OC, GR, W], f32)
                pp = ppA if g % 2 == 0 else ppB
                for sp in range(SPE // 2):
                    # two phases (2*sp, 2*sp+1) share one 2-bank psum tile so
                    # the evacuation is a single wider DVE copy
                    ps = pp.tile([OC, 2, GR, 16], f32, tag="ps")
                    for b in range(2):
                        s = 2 * sp + b
                        for dj in range(3):
                            nc.tensor.matmul(
                                ps[:, b],
                                w_sb[:, s, dj, :],
                                x3[:, :, s + dj : s + dj + 241 : 16],
                                start=(dj == 0),
                                stop=(dj == 2),
                            )
                    # dst covers w columns {w16*16 + 2*sp + b}; iteration
                    # order (oc, b, h, w16) matches the contiguous psum src
                    dst = o_sb[:, :, 2 * sp : 2 * sp + 241 : 16]
                    dst = dst.unsqueeze(1).broadcast_to([OC, 2, GR, 16])
                    dst.ap = bass_rust.VecI64Pair(
                        [
                            [GR * W, OC],
                            [1, 2],
                            [W, GR],
                            [16, 16],
                        ]
                    )
                    nc.vector.tensor_copy(dst, ps[:])
                nc.scalar.dma_start(out[:, g * GR : (g + 1) * GR, :], o_sb[:])
    nc.finalize()
    return nc


def shard_inputs(x, weight):
    x = np.ascontiguousarray(np.asarray(x), dtype=np.float32)
    weight = np.ascontiguousarray(np.asarray(weight), dtype=np.float32)
    xp = np.zeros((N, C, H + 2, W + 2), np.float32)
    xp[:, :, 1:-1, 1:-1] = x
    # wh[(di*32+c), s, dj, oc] = weight[s, c*9+di*3+dj, oc]
    wh = np.ascontiguousarray(
        weight.reshape(SPE, C, 3, 3, OC).transpose(2, 1, 0, 3, 4)
    ).reshape(KDIM, SPE, 3, OC)
    in_maps = []
    for k in range(NCORES):
        n, r0 = k // 2, (k % 2) * ROWS
        in_maps.append(
            {"xin": np.ascontiguousarray(xp[n, :, r0 : r0 + ROWS + 2, :]), "w": wh}
        )
    return in_maps


def unshard_outputs(results):
    out = np.empty((N, OC, H, W), np.float32)
    for k in range(NCORES):
        n, r0 = k // 2, (k % 2) * ROWS
        out[n, :, r0 : r0 + ROWS, :] = results[k]["out"]
    return out


def run(x, weight, **spmd_kwargs):
    from concourse.bass_utils import run_bass_kernel_spmd

    in_maps = shard_inputs(x, weight)
    if "nc" not in _cache:
        _cache["nc"] = build_nc()
    res = run_bass_kernel_spmd(_cache["nc"], in_maps, list(range(NCORES)), **spmd_kwargs)
    return unshard_outputs(res.results), res


def kernel(x, weight):
    out, _ = run(x, weight)
    return out



# revision 4
# speedup vs baseline: 1.8308x; 1.8308x over previous
"""Trainium2 Bass kernel for ConvMosaic: 3x3 conv (pad 1) where the weight set
depends on output position p%16 == w%16 (column phase).

Strategy (8 NeuronCores, SPMD):
  - Shard over (N, H): core k handles image k//2, row-half k%2 (128 rows).
  - Everything bf16 on the wire (x, weights, output); accumulate f32 in PSUM.
  - Host pre-pads each core chunk to [C=32, 130, 258] bf16.
  - On-chip x staged as 3 row-shifted partition blocks [(c,di)=96, 32h, 258w]
    (c-major so the input DMA's outermost dim is 32 -> spreads across all 16
    SDMA engines; di-major only used 3 engines = ~67 GB/s).
  - Phase pairing: weights of phases (2pi, 2pi+1) stacked along M=128. Taps
    (p=0, dj=j) and (p=1, dj=j-1) read the same x columns, so each pair is
    4 accumulating matmuls [96,128]x[96,32,16] (j=0..3) into one PSUM tile
    [128 = (p,oc), 32h, 16w16]; 25% of weight entries are structural zeros.
  - Two pairs share a 2-bank PSUM tile; evacuation = 2 copies (p=0 -> DVE,
    p=1 -> ACT) of [64, 1024] into partition-matched halves of a
    [128=(p,oc), 32h, 128w2] bf16 staging tile (w2 = w//2: even/odd output
    columns live on separate partition halves; host de-interleaves).
  - Output DMA per group: [128, 32, 128] bf16, 8KB contiguous runs per
    partition.
"""

import sys

import numpy as np

for _p in ("/opt/trn_rl_repo",):
    if _p not in sys.path:
        sys.path.insert(0, _p)

N, C, H, W = 4, 32, 256, 256
OC = 64
SPE = 16
KDIM = 96  # (c, di) contraction partitions
NCORES = 8
ROWS = H * N // NCORES  # 128 rows per core
GR = 32  # rows per group
GROUPS = ROWS // GR
NPAIR = SPE // 2  # 8 phase pairs

_cache = {}


def build_nc():
    import bass_rust
    from concourse import bacc, bass, mybir, tile

    f32 = mybir.dt.float32
    bf16 = mybir.dt.bfloat16

    nc = bacc.Bacc()
    xin = nc.dram_tensor("xin", [C, ROWS + 2, W + 2], bf16, kind="ExternalInput")
    wdr = nc.dram_tensor("w", [KDIM, NPAIR, 4, 2 * OC], bf16, kind="ExternalInput")
    out = nc.dram_tensor("out", [2 * OC, ROWS, W // 2], bf16, kind="ExternalOutput")

    with tile.TileContext(nc) as tc:
        with (
            tc.tile_pool(name="wp", bufs=1) as wp,
            tc.tile_pool(name="xp", bufs=3) as xp,
            tc.tile_pool(name="op", bufs=2) as op,
            tc.tile_pool(name="pp", bufs=2, space=bass.MemorySpace.PSUM) as pp,
            tc.tile_pool(name="ppw", bufs=1, space=bass.MemorySpace.PSUM) as ppw,
        ):
            w_sb = wp.tile([KDIM, NPAIR, 4, 2 * OC], bf16)

            x3s = []
            for g in range(GROUPS):
                x3 = xp.tile([KDIM, GR, W + 2], bf16)
                # single DMA: src AP iterates (c, di, h, ww) with overlapping
                # row windows (di stride = 1 row) -> fills all 96 partitions.
                # c outermost so descriptors spread over all 16 SDMA engines.
                src = xin[:, g * GR : g * GR + GR + 2, :]
                src = src.unsqueeze(1).broadcast_to([C, 3, GR + 2, W + 2])
                src.ap = bass_rust.VecI64Pair(
                    [
                        [(ROWS + 2) * (W + 2), C],
                        [W + 2, 3],
                        [W + 2, GR],
                        [1, W + 2],
                    ]
                )
                if g == 0:
                    nc.sync.dma_start(x3[:], src)
                    nc.sync.dma_start(w_sb[:], wdr[:])
                    # dummy matmul: absorbs the weight-DMA wait so the first
                    # real matmul carries only one wait.
                    scratch = ppw.tile([2 * OC, 16], f32)
                    nc.tensor.matmul(
                        scratch[:],
                        w_sb[:, 0, 0, :],
                        w_sb[:, 0, 0, 0:16],
                        start=True,
                        stop=True,
                    )
                else:
                    nc.sync.dma_start(x3[:], src)
                x3s.append(x3)

            for g in range(GROUPS):
                x3 = x3s[g]
                o_sb = op.tile([2 * OC, GR, W // 2], bf16)
                for pq in range(NPAIR // 2):
                    # two phase-pairs (2pq, 2pq+1) share one 2-bank psum tile
                    ps = pp.tile([2 * OC, 2, GR, 16], f32, tag="ps")
                    for b in range(2):
                        pi = 2 * pq + b
                        s0 = 2 * pi
                        for j in range(4):
                            nc.tensor.matmul(
                                ps[:, b],
                                w_sb[:, pi, j, :],
                                x3[:, :, s0 + j : s0 + j + 241 : 16],
                                start=(j == 0),
                                stop=(j == 3),
                            )
                    # evacuate: phase p lives on psum partitions p*64..p*64+63
                    # and lands on the same partitions of o_sb; free-dim
                    # mapping (b, h, w16) -> offset 2*pq + b + 128*h + 8*w16.
                    for p in range(2):
                        src_ps = ps[p * OC : (p + 1) * OC, :]
                        dst = o_sb[
                            p * OC : (p + 1) * OC, :, 2 * pq : 2 * pq + 121 : 8
                        ]
                        dst = dst.unsqueeze(1).broadcast_to([OC, 2, GR, 16])
                        dst.ap = bass_rust.VecI64Pair(
                            [
                                [GR * (W // 2), OC],
                                [1, 2],
                                [W // 2, GR],
                                [8, 16],
                            ]
                        )
                        if p == 0:
                            nc.vector.tensor_copy(dst, src_ps)
                        else:
                            nc.scalar.copy(dst, src_ps)
                nc.scalar.dma_start(out[:, g * GR : (g + 1) * GR, :], o_sb[:])
    nc.finalize()
    return nc


def shard_inputs(x, weight):
    import ml_dtypes

    bf16 = ml_dtypes.bfloat16
    x = np.asarray(x, dtype=np.float32)
    weight = np.asarray(weight, dtype=np.float32)
    xp = np.zeros((N, C, H + 2, W + 2), bf16)
    xp[:, :, 1:-1, 1:-1] = x.astype(bf16)
    # w4[(c*3+di), pi, j, p*64+oc] = weight[2pi+p, c*9+di*3+(j-p), oc]
    # (zero when j-p outside [0,3))
    w5 = weight.reshape(SPE, C, 3, 3, OC)  # [s, c, di, dj, oc]
    w4 = np.zeros((C, 3, NPAIR, 4, 2, OC), np.float32)  # [c, di, pi, j, p, oc]
    for p in range(2):
        for j in range(4):
            dj = j - p
            if 0 <= dj < 3:
                # w5[p::2] -> [pi, c, di, oc]
                w4[:, :, :, j, p, :] = w5[p::2, :, :, dj, :].transpose(1, 2, 0, 3)
    wh = np.ascontiguousarray(w4.reshape(KDIM, NPAIR, 4, 2 * OC).astype(bf16))
    in_maps = []
    for k in range(NCORES):
        n, r0 = k // 2, (k % 2) * ROWS
        in_maps.append(
            {"xin": np.ascontiguousarray(xp[n, :, r0 : r0 + ROWS + 2, :]), "w": wh}
        )
    return in_maps


def unshard_outputs(results):
    out = np.empty((N, OC, H, W), np.float32)
    for k in range(NCORES):
        n, r0 = k // 2, (k % 2) * ROWS
        r = np.asarray(results[k]["out"]).astype(np.float32)
        r = r.reshape(2, OC, ROWS, W // 2)
        out[n, :, r0 : r0 + ROWS, 0::2] = r[0]
        out[n, :, r0 : r0 + ROWS, 1::2] = r[1]
    return out


def run(x, weight, **spmd_kwargs):
    from concourse.bass_utils import run_bass_kernel_spmd

    in_maps = shard_inputs(x, weight)
    if "nc" not in _cache:
        _cache["nc"] = build_nc()
    res = run_bass_kernel_spmd(_cache["nc"], in_maps, list(range(NCORES)), **spmd_kwargs)
    return unshard_outputs(res.results), res


def kernel(x, weight):
    out, _ = run(x, weight)
    return out


# revision 5
# speedup vs baseline: 4.7282x; 2.5826x over previous
"""Trainium2 Bass kernel for ConvMosaic: 3x3 conv (pad 1) where the weight set
depends on output position p%16 == w%16 (column phase).

Strategy (8 NeuronCores, SPMD):
  - Shard over (N, H): core k handles image k//2, row-half k%2 (128 rows).
  - Everything bf16 on the wire; f32 accumulation in PSUM.
  - Host de-interleaves x columns by phase class t = w%16: device x layout is
    [C, rows, t=18, k=16] so every matmul rhs is rows of 16 CONTIGUOUS
    elements (strided moving operands run 3x slower on the PE).
  - On-chip x staged as 3 row-shifted partition blocks [(c,di)=96, 32h, 288]
    via one DMA with overlapping row windows; c-major so the DMA spreads
    over all 16 SDMA engines.
  - Phase pairing: weights of phases (2pi, 2pi+1) stacked along M=128; taps
    (p=0, dj=j) and (p=1, dj=j-1) share the rhs slice t = 2pi+j, so each
    pair is 4 accumulating matmuls [96,128]x[96,32,16] into one PSUM tile
    [128=(p,oc), 32h, 16k].
  - Two pairs share a 2-bank PSUM tile; evacuation is a single fully
    contiguous [128, 1024] f32->bf16 copy (alternating DVE/ACT) into
    T[128, pair, h, k]; output DMA is contiguous 8KB runs. The host undoes
    the (w16, pair, phase) -> w interleave with one numpy transpose.
"""

import sys

import numpy as np

for _p in ("/opt/trn_rl_repo",):
    if _p not in sys.path:
        sys.path.insert(0, _p)

N, C, H, W = 4, 32, 256, 256
OC = 64
SPE = 16
KDIM = 96  # (c, di) contraction partitions
NCORES = 8
ROWS = H * N // NCORES  # 128 rows per core
GR = 32  # rows per group
GROUPS = ROWS // GR
NPAIR = SPE // 2  # 8 phase pairs
TCLS = 18  # de-interleaved column classes (s0+j reaches 14+3=17)
XW = TCLS * 16  # 288 columns per de-interleaved row

_cache = {}


def build_nc():
    import bass_rust
    from concourse import bacc, bass, mybir, tile

    f32 = mybir.dt.float32
    bf16 = mybir.dt.bfloat16

    nc = bacc.Bacc()
    xin = nc.dram_tensor("xin", [C, ROWS + 2, XW], bf16, kind="ExternalInput")
    wdr = nc.dram_tensor("w", [KDIM, NPAIR, 4, 2 * OC], bf16, kind="ExternalInput")
    out = nc.dram_tensor(
        "out", [2 * OC, GROUPS, NPAIR, GR, 16], bf16, kind="ExternalOutput"
    )

    with tile.TileContext(nc) as tc:
        with (
            tc.tile_pool(name="wp", bufs=1) as wp,
            tc.tile_pool(name="xp", bufs=3) as xp,
            tc.tile_pool(name="op", bufs=2) as op,
            tc.tile_pool(name="pp", bufs=2, space=bass.MemorySpace.PSUM) as pp,
            tc.tile_pool(name="ppw", bufs=1, space=bass.MemorySpace.PSUM) as ppw,
        ):
            w_sb = wp.tile([KDIM, NPAIR, 4, 2 * OC], bf16)

            x3s = []
            for g in range(GROUPS):
                x3 = xp.tile([KDIM, GR, XW], bf16)
                # single DMA: src AP iterates (c, di, h, tk) with overlapping
                # row windows (di stride = 1 row) -> fills all 96 partitions.
                # c outermost spreads descriptors over all 16 SDMA engines.
                src = xin[:, g * GR : g * GR + GR + 2, :]
                src = src.unsqueeze(1).broadcast_to([C, 3, GR + 2, XW])
                src.ap = bass_rust.VecI64Pair(
                    [
                        [(ROWS + 2) * XW, C],
                        [XW, 3],
                        [XW, GR],
                        [1, XW],
                    ]
                )
                if g == 0:
                    nc.sync.dma_start(x3[:], src)
                    nc.sync.dma_start(w_sb[:], wdr[:])
                    # dummy matmul: absorbs the weight-DMA wait so the first
                    # real matmul carries only one wait.
                    scratch = ppw.tile([2 * OC, 16], f32)
                    nc.tensor.matmul(
                        scratch[:],
                        w_sb[:, 0, 0, :],
                        w_sb[:, 0, 0, 0:16],
                        start=True,
                        stop=True,
                    )
                else:
                    nc.sync.dma_start(x3[:], src)
                x3s.append(x3)

            for g in range(GROUPS):
                x3 = x3s[g]
                o_sb = op.tile([2 * OC, NPAIR, GR, 16], bf16)
                for pq in range(NPAIR // 2):
                    # two phase-pairs (2pq, 2pq+1) share one 2-bank psum tile
                    ps = pp.tile([2 * OC, 2, GR, 16], f32, tag="ps")
                    for b in range(2):
                        pi = 2 * pq + b
                        s0 = 2 * pi
                        for j in range(4):
                            t = s0 + j
                            nc.tensor.matmul(
                                ps[:, b],
                                w_sb[:, pi, j, :],
                                x3[:, :, 16 * t : 16 * (t + 1)],
                                start=(j == 0),
                                stop=(j == 3),
                            )
                    # fully contiguous evacuation [128, 1024], DVE/ACT split
                    dst = o_sb[:, 2 * pq : 2 * pq + 2]
                    if pq % 2 == 0:
                        nc.vector.tensor_copy(dst, ps[:])
                    else:
                        nc.scalar.copy(dst, ps[:])
                nc.scalar.dma_start(out[:, g], o_sb[:])
    nc.finalize()
    return nc


def shard_inputs(x, weight):
    import ml_dtypes

    bf16 = ml_dtypes.bfloat16
    x = np.asarray(x, dtype=np.float32)
    weight = np.asarray(weight, dtype=np.float32)
    xpad = np.zeros((N, C, H + 2, W + 2), np.float32)
    xpad[:, :, 1:-1, 1:-1] = x
    # de-interleave columns: xcls[..., t, k] = xpad[..., 16k + t]
    xcls = np.empty((N, C, H + 2, TCLS, 16), bf16)
    for t in range(TCLS):
        xcls[..., t, :] = xpad[..., t : t + 241 : 16].astype(bf16)
    xcls = xcls.reshape(N, C, H + 2, XW)
    # w4[(c*3+di), pi, j, p*64+oc] = weight[2pi+p, c*9+di*3+(j-p), oc]
    w5 = weight.reshape(SPE, C, 3, 3, OC)  # [s, c, di, dj, oc]
    w4 = np.zeros((C, 3, NPAIR, 4, 2, OC), np.float32)  # [c, di, pi, j, p, oc]
    for p in range(2):
        for j in range(4):
            dj = j - p
            if 0 <= dj < 3:
                w4[:, :, :, j, p, :] = w5[p::2, :, :, dj, :].transpose(1, 2, 0, 3)
    wh = np.ascontiguousarray(w4.reshape(KDIM, NPAIR, 4, 2 * OC).astype(bf16))
    in_maps = []
    for k in range(NCORES):
        n, r0 = k // 2, (k % 2) * ROWS
        in_maps.append(
            {"xin": np.ascontiguousarray(xcls[n, :, r0 : r0 + ROWS + 2, :]), "w": wh}
        )
    return in_maps


def unshard_outputs(results):
    out = np.empty((N, OC, H, W), np.float32)
    for k in range(NCORES):
        n, r0 = k // 2, (k % 2) * ROWS
        r = np.asarray(results[k]["out"]).astype(np.float32)
        r = r.reshape(2, OC, GROUPS, NPAIR, GR, 16)
        # out[oc, g*GR+h, 16*k + 2*pi + p] = r[p, oc, g, pi, h, k]
        r = r.transpose(1, 2, 4, 5, 3, 0)  # [oc, g, h, k, pi, p]
        out[n, :, r0 : r0 + ROWS, :] = r.reshape(OC, ROWS, W)
    return out


def run(x, weight, **spmd_kwargs):
    from concourse.bass_utils import run_bass_kernel_spmd

    in_maps = shard_inputs(x, weight)
    if "nc" not in _cache:
        _cache["nc"] = build_nc()
    res = run_bass_kernel_spmd(_cache["nc"], in_maps, list(range(NCORES)), **spmd_kwargs)
    return unshard_outputs(res.results), res


def kernel(x, weight):
    out, _ = run(x, weight)
    return out
